# revision 1
# baseline (speedup 1.0000x reference)
"""Trainium2 Bass kernel for nn_ConditionalFeaturesUpsample.

Reference computation (B=1, L=64, C=80):
    x   = local_features[0].T                          # [80, 64]
    up  = ConvTranspose1d(x; wt, bt, k=stride=4)       # [80, 256]
    y   = w1 @ up + b1                                 # [3072, 256]
    out = tile(y, 75) reshaped to [128, 1, 24, 19200]  # out[ch,0,l,t] = y[l*128+ch, t%256]

Sharding: tensor-parallel over the 3072 output channels (batch is 1).
Core i computes channel rows {l*128 + 16*i + j}, i.e. the slice
out[16*i:16*(i+1), 0, :, :]; the host gather is a concat + transpose.

Host-side weight preprocessing (pure algebra, no activations touched):
    W2[m,c,k] = sum_o w1[m,o] * wt[c,o,k]   (ConvT folded into the 1x1 conv)
    b_eff     = w1 @ bt + b1
so each core runs 12 matmuls [80 -> 128, 64] straight from x, then adds
b_eff while rearranging PSUM [m,(k,l)] -> SBUF [m, 4l+k].

The 75x time-repeat is never materialized: a [128, 768] tile (3 periods)
is built per group and two DMA instructions per group write the whole
19200-wide span using a zero-stride (broadcast) source AP -> 3 KB
descriptors, measured at ~353 GB/s (98.6% of the ~358 GB/s per-core HBM
write wall). Weights ship in two packed tensors so group 0's matmuls
start as soon as the first (smaller) DMA lands.
"""
import os
import sys

import numpy as np

for _p in ("/opt/trn_rl_repo", "/root/.axon_site/_ro/trn_rl_repo"):
    if os.path.isdir(_p) and _p not in sys.path:
        sys.path.append(_p)

import concourse.bacc as bacc
import concourse.mybir as mybir
import concourse.tile as tile
from concourse.bass_utils import run_bass_kernel_spmd

UPSAMPLE_REPEAT = 75
NUM_LAYERS = 24
N_CORES = 8
GROUPS = 3             # groups of 128 channel-rows per core
T_SMALL = 256
T_FULL = T_SMALL * UPSAMPLE_REPEAT  # 19200
F32 = mybir.dt.float32

CHUNK = 768            # 3 periods per broadcast-source tile
SPANS = ((0, 21), (16128, 4))  # (elem offset, CHUNK-repeats): 21*768 + 4*768 = 19200

# par1 [128, 579]: [0:3) b_eff | [3:67) x | [67:579) W2 g0 (4 lhsT chunks of 128)
# par2 [128, 1024]: W2 g1, g2 (8 lhsT chunks of 128)
P1_BE, P1_X, P1_W2, P1_COLS = 0, 3, 67, 579
P2_COLS = 1024


def build_bass():
    nc = bacc.Bacc()
    par1_d = nc.declare_dram_parameter("par1", [128, P1_COLS], F32, isOutput=False)
    par2_d = nc.declare_dram_parameter("par2", [128, P2_COLS], F32, isOutput=False)
    # l-major per-core output: out[l, j, t] = y[(8g+l)*128 + 16*core + j, t%256]
    out_d = nc.declare_dram_parameter("out", [NUM_LAYERS, 16, T_FULL], F32, isOutput=True)

    with tile.TileContext(nc) as tc:
        with (
            tc.tile_pool(name="consts", bufs=1) as consts,
            tc.tile_pool(name="psum", bufs=2, space="PSUM") as psum_pool,
            tc.tile_pool(name="mid", bufs=3) as mid_pool,
        ):
            par1_sb = consts.tile([128, P1_COLS], F32)
            nc.sync.dma_start(out=par1_sb[:], in_=par1_d[:])
            par2_sb = consts.tile([128, P2_COLS], F32)
            nc.sync.dma_start(out=par2_sb[:], in_=par2_d[:])
            be_sb = par1_sb[:, P1_BE:P1_X]
            x_sb = par1_sb[0:80, P1_X:P1_W2]

            def w2chunk(g, k):
                if g == 0:
                    return par1_sb[0:80, P1_W2 + 128 * k:P1_W2 + 128 * (k + 1)]
                off = 128 * (4 * (g - 1) + k)
                return par2_sb[0:80, off:off + 128]

            for g in range(GROUPS):
                y_ps = psum_pool.tile([128, T_SMALL], F32, tag="y_ps")
                for k in range(4):
                    nc.tensor.matmul(
                        y_ps[:, 64 * k:64 * (k + 1)],
                        lhsT=w2chunk(g, k),
                        rhs=x_sb,
                        start=True,
                        stop=True,
                    )
                y_mid = mid_pool.tile([128, CHUNK], F32, tag="y_mid")
                # PSUM [m,(k,l)] -> SBUF [m, 4l+k] with per-partition bias add
                nc.scalar.activation(
                    out=y_mid[:, :T_SMALL].rearrange("p (l k) -> p k l", k=4),
                    in_=y_ps[:].rearrange("p (k l) -> p k l", k=4),
                    func=mybir.ActivationFunctionType.Identity,
                    bias=be_sb[:, g:g + 1],
                )
                # Fill the remaining 2 periods by doubling
                filled = T_SMALL
                while filled < CHUNK:
                    n = min(filled, CHUNK - filled)
                    nc.vector.tensor_copy(
                        out=y_mid[:, filled:filled + n], in_=y_mid[:, :n]
                    )
                    filled += n
                # Two broadcast-source DMAs per group write all 75 periods;
                # group rows (l,j) are contiguous in the l-major layout.
                grp = out_d[8 * g:8 * (g + 1), :, :].rearrange("l j t -> (l j) t")
                for start, reps in SPANS:
                    nc.sync.dma_start(
                        out=grp[:, start:start + reps * CHUNK],
                        in_=y_mid[:].unsqueeze(1).broadcast_to([128, reps, CHUNK]),
                    )
    nc.compile()
    return nc


def host_prep(local_features, wt, bt, w1, b1):
    lf = np.asarray(local_features, np.float32)
    wt64 = np.asarray(wt, np.float64)
    w164 = np.asarray(w1, np.float64)
    x = lf[0].T.astype(np.float32)                           # [80, 64]
    W2 = np.einsum('mo,cok->mck', w164, wt64).astype(np.float32)  # [3072,80,4]
    b_eff = (w164 @ np.asarray(bt, np.float64)
             + np.asarray(b1, np.float64)).astype(np.float32)

    # Channel row for (core, g, p): c = (8g + p//16)*128 + 16*core + p%16
    g_idx = np.arange(GROUPS)[:, None]
    p_idx = np.arange(128)[None, :]
    base = (8 * g_idx + p_idx // 16) * 128 + p_idx % 16      # l-major partitions
    in_maps = []
    for core in range(N_CORES):
        c = base + 16 * core                                 # [3, 128]
        W2sel = W2[c]                                        # [3, 128, 80, 4]
        par1 = np.zeros((128, P1_COLS), np.float32)
        par1[:, P1_BE:P1_X] = b_eff[c].T
        par1[0:80, P1_X:P1_W2] = x
        par1[0:80, P1_W2:] = np.concatenate(
            [W2sel[0, :, :, k].T for k in range(4)], axis=1)
        par2 = np.zeros((128, P2_COLS), np.float32)
        par2[0:80, :] = np.concatenate(
            [W2sel[g, :, :, k].T for g in (1, 2) for k in range(4)], axis=1)
        in_maps.append({"par1": par1, "par2": par2})
    return in_maps


def run(inputs, trace=False, **spmd_kwargs):
    """Returns (full_output [128,1,24,19200], BassKernelResults)."""
    nc = build_bass()
    in_maps = host_prep(**inputs)
    res = run_bass_kernel_spmd(
        nc, in_maps, core_ids=list(range(N_CORES)), trace=trace, **spmd_kwargs
    )
    out = np.empty((128, 1, NUM_LAYERS, T_FULL), np.float32)
    for i in range(N_CORES):
        shard = np.asarray(res.results[i]["out"])    # [24, 16, 19200]
        out[16 * i:16 * (i + 1), 0] = shard.transpose(1, 0, 2)
    return out, res


def kernel(**inputs):
    out, _ = run(inputs, trace=False)
    return out



# revision 2
# speedup vs baseline: 1.0463x; 1.0463x over previous
"""Trainium2 Bass kernel for nn_ConditionalFeaturesUpsample.

Reference computation (B=1, L=64, C=80):
    x   = local_features[0].T                          # [80, 64]
    up  = ConvTranspose1d(x; wt, bt, k=stride=4)       # [80, 256]
    y   = w1 @ up + b1                                 # [3072, 256]
    out = tile(y, 75) reshaped to [128, 1, 24, 19200]  # out[ch,0,l,t] = y[l*128+ch, t%256]

Sharding: tensor-parallel over the 3072 output channels (batch is 1).
Core i computes channel rows {l*128 + 16*i + j}, i.e. the slice
out[16*i:16*(i+1), 0, :, :]; the host gather is a concat + transpose.

Host-side weight preprocessing (pure algebra, no activations touched):
    W2[m,c,k] = sum_o w1[m,o] * wt[c,o,k]   (ConvT folded into the 1x1 conv)
    b_eff     = w1 @ bt + b1
The bias is folded into the matmul itself: x gains a row of ones and each
lhsT chunk gains a b_eff row (contraction 80 -> 81), so PSUM already holds
y + b and the scalar-engine activation (and its 1.5 us ACT_TABLE_LOAD) is
off the critical path entirely.  PSUM [m,(k,l)] -> SBUF [m, 4l+k] is a
plain DVE copy.

The 75x time-repeat is never materialized in full: per group a
[128, 3840] tile (15 periods) is built by DVE doubling and broadcast
(zero-stride) source DMAs write the 19200-wide span with 15 KB
descriptors (vs 3 KB before), lifting the SDMA engines from ~86% to
~99% of line rate.  Group 0 additionally writes its first 768 columns
as soon as they exist so the engines start ~3 us earlier.  Weights ship
in two packed tensors so group 0's matmuls start as soon as the first
(smaller) DMA lands.
"""
import os
import sys

import numpy as np

for _p in ("/opt/trn_rl_repo", "/root/.axon_site/_ro/trn_rl_repo"):
    if os.path.isdir(_p) and _p not in sys.path:
        sys.path.append(_p)

import concourse.bacc as bacc
import concourse.mybir as mybir
import concourse.tile as tile
from concourse.bass_utils import run_bass_kernel_spmd

UPSAMPLE_REPEAT = 75
NUM_LAYERS = 24
N_CORES = 8
GROUPS = 3             # groups of 128 channel-rows per core
T_SMALL = 256
T_FULL = T_SMALL * UPSAMPLE_REPEAT  # 19200
F32 = mybir.dt.float32

CHUNK = 3840           # 15 periods per broadcast-source tile; 19200 = 5*3840
HEAD = 768             # group 0's early-start write width

# parA [81, 576]: [0:64) x_aug | [64:576) W2 g0 (4 lhsT chunks of 128)
# parB [81, 1024]: W2 g1, g2 (8 lhsT chunks of 128)
PA_X, PA_W2, PA_COLS = 0, 64, 576
PB_COLS = 1024
KDIM = 81              # 80 channels + ones/bias row


def build_bass():
    nc = bacc.Bacc()
    parA_d = nc.declare_dram_parameter("parA", [KDIM, PA_COLS], F32, isOutput=False)
    parB_d = nc.declare_dram_parameter("parB", [KDIM, PB_COLS], F32, isOutput=False)
    # l-major per-core output: out[l, j, t] = y[(8g+l)*128 + 16*core + j, t%256]
    out_d = nc.declare_dram_parameter("out", [NUM_LAYERS, 16, T_FULL], F32, isOutput=True)

    with tile.TileContext(nc) as tc:
        with (
            tc.tile_pool(name="consts", bufs=1) as consts,
            tc.tile_pool(name="psum", bufs=2, space="PSUM") as psum_pool,
            tc.tile_pool(name="mid", bufs=3) as mid_pool,
        ):
            parA_sb = consts.tile([KDIM, PA_COLS], F32)
            nc.sync.dma_start(out=parA_sb[:], in_=parA_d[:])
            parB_sb = consts.tile([KDIM, PB_COLS], F32)
            nc.sync.dma_start(out=parB_sb[:], in_=parB_d[:])
            x_sb = parA_sb[:, PA_X:PA_W2]

            def w2chunk(g, k):
                if g == 0:
                    return parA_sb[:, PA_W2 + 128 * k:PA_W2 + 128 * (k + 1)]
                off = 128 * (4 * (g - 1) + k)
                return parB_sb[:, off:off + 128]

            for g in range(GROUPS):
                y_ps = psum_pool.tile([128, T_SMALL], F32, tag="y_ps")
                for k in range(4):
                    nc.tensor.matmul(
                        y_ps[:, 64 * k:64 * (k + 1)],
                        lhsT=w2chunk(g, k),
                        rhs=x_sb,
                        start=True,
                        stop=True,
                    )
                y_mid = mid_pool.tile([128, CHUNK], F32, tag="y_mid")
                # PSUM [m,(k,l)] -> SBUF [m, 4l+k]; bias already in PSUM
                nc.vector.tensor_copy(
                    out=y_mid[:, :T_SMALL].rearrange("p (l k) -> p k l", k=4),
                    in_=y_ps[:].rearrange("p (k l) -> p k l", k=4),
                )
                grp = out_d[8 * g:8 * (g + 1), :, :].rearrange("l j t -> (l j) t")
                # Fill to HEAD, kick off the head write (group 0 only: the
                # engines are idle until the first output DMA anyway), then
                # keep doubling to the full CHUNK.
                filled = T_SMALL
                while filled < HEAD:
                    n = min(filled, HEAD - filled)
                    nc.vector.tensor_copy(
                        out=y_mid[:, filled:filled + n], in_=y_mid[:, :n]
                    )
                    filled += n
                if g == 0:
                    nc.sync.dma_start(out=grp[:, :HEAD], in_=y_mid[:, :HEAD])
                while filled < CHUNK:
                    n = min(filled, CHUNK - filled)
                    nc.vector.tensor_copy(
                        out=y_mid[:, filled:filled + n], in_=y_mid[:, :n]
                    )
                    filled += n
                if g == 0:
                    # [HEAD:CHUNK) once, then 4 broadcast repeats of CHUNK
                    nc.sync.dma_start(
                        out=grp[:, HEAD:CHUNK], in_=y_mid[:, HEAD:CHUNK]
                    )
                    nc.sync.dma_start(
                        out=grp[:, CHUNK:],
                        in_=y_mid[:].unsqueeze(1).broadcast_to([128, 4, CHUNK]),
                    )
                else:
                    nc.sync.dma_start(
                        out=grp[:],
                        in_=y_mid[:].unsqueeze(1).broadcast_to([128, 5, CHUNK]),
                    )
    nc.compile()
    return nc


def host_prep(local_features, wt, bt, w1, b1):
    lf = np.asarray(local_features, np.float32)
    wt64 = np.asarray(wt, np.float64)
    w164 = np.asarray(w1, np.float64)
    x = lf[0].T.astype(np.float32)                           # [80, 64]
    W2 = np.einsum('mo,cok->mck', w164, wt64).astype(np.float32)  # [3072,80,4]
    b_eff = (w164 @ np.asarray(bt, np.float64)
             + np.asarray(b1, np.float64)).astype(np.float32)

    # Channel row for (core, g, p): c = (8g + p//16)*128 + 16*core + p%16
    g_idx = np.arange(GROUPS)[:, None]
    p_idx = np.arange(128)[None, :]
    base = (8 * g_idx + p_idx // 16) * 128 + p_idx % 16      # l-major partitions
    in_maps = []
    for core in range(N_CORES):
        c = base + 16 * core                                 # [3, 128]
        W2sel = W2[c]                                        # [3, 128, 80, 4]
        bsel = b_eff[c]                                      # [3, 128]

        def lhsT(g, k):
            # [81, 128]: rows 0..79 weights, row 80 bias (pairs with ones row)
            m = np.empty((KDIM, 128), np.float32)
            m[:80] = W2sel[g, :, :, k].T
            m[80] = bsel[g]
            return m

        parA = np.zeros((KDIM, PA_COLS), np.float32)
        parA[:80, PA_X:PA_W2] = x
        parA[80, PA_X:PA_W2] = 1.0
        parA[:, PA_W2:] = np.concatenate([lhsT(0, k) for k in range(4)], axis=1)
        parB = np.concatenate(
            [lhsT(g, k) for g in (1, 2) for k in range(4)], axis=1)
        in_maps.append({"parA": parA, "parB": np.ascontiguousarray(parB)})
    return in_maps


def run(inputs, trace=False, **spmd_kwargs):
    """Returns (full_output [128,1,24,19200], BassKernelResults)."""
    nc = build_bass()
    in_maps = host_prep(**inputs)
    res = run_bass_kernel_spmd(
        nc, in_maps, core_ids=list(range(N_CORES)), trace=trace, **spmd_kwargs
    )
    out = np.empty((128, 1, NUM_LAYERS, T_FULL), np.float32)
    for i in range(N_CORES):
        shard = np.asarray(res.results[i]["out"])    # [24, 16, 19200]
        out[16 * i:16 * (i + 1), 0] = shard.transpose(1, 0, 2)
    return out, res


def kernel(**inputs):
    out, _ = run(inputs, trace=False)
    return out
